# revision 5
# baseline (speedup 1.0000x reference)
"""Grayscale + single-level 2x2 Haar DWT kernel for Trainium2 (8 cores, SPMD).

Full input x [16,3,1024,1024] f32 -> out [16,4,512,512] f32.
Batch-sharded: core i handles samples [2i, 2i+1].

Math per sample (BGR weights w=(0.114,0.587,0.299), all bands scaled by 0.5):
  gray = w0*x[0] + w1*x[1] + w2*x[2]
  a,b,c,d = gray[0::2,0::2], gray[0::2,1::2], gray[1::2,0::2], gray[1::2,1::2]
  cA,cH,cV,cD = 0.5*(a+b+c+d), 0.5*(a+b-c-d), 0.5*(a-b+c-d), 0.5*(a-b-c+d)

Per band of 128 output rows (= 256 input rows), all ops in-place where legal:
  E_ch/O_ch = even/odd input rows [128,1024] via strided DMA
  E0 += r1*E1 ; E0 += r2*E2  (r_i = w_i/w_0)    - scalar_tensor_tensor on DVE
  E0 *= w0/2   (ACT engine)                      - same for O0
  drow = E0 - O0 ; E0 += O0 (=srow)              - tensor_tensor on DVE
  cA/cV = E0[:,0::2] +/- E0[:,1::2] ; cH/cD = drow[:,0::2] +/- drow[:,1::2]
"""

import numpy as np

N_CORES = 8
B, C, H, W = 16, 3, 1024, 1024
HO, WO = H // 2, W // 2
SPC = B // N_CORES  # samples per core

W_BGR = (0.114, 0.587, 0.299)

_compiled = None


def _build():
    from concourse import bacc, mybir
    from concourse.tile import TileContext

    f32 = mybir.dt.float32
    add = mybir.AluOpType.add
    sub = mybir.AluOpType.subtract
    mult = mybir.AluOpType.mult

    w0, w1, w2 = W_BGR
    r1 = w1 / w0
    r2 = w2 / w0
    w0h = w0 * 0.5

    nc = bacc.Bacc("TRN2", target_bir_lowering=False, debug=False)
    x = nc.declare_dram_parameter("x", [SPC, C, H, W], f32, isOutput=False)
    out = nc.declare_dram_parameter("out", [SPC, 4, HO, WO], f32, isOutput=True)

    n_bands = H // 256  # bands of 128 output rows per sample

    with TileContext(nc) as tc:
        with (
            tc.tile_pool(name="in_pool", bufs=4) as in_pool,
            tc.tile_pool(name="mid_pool", bufs=4) as mid_pool,
            tc.tile_pool(name="out_pool", bufs=4) as out_pool,
        ):
            for s in range(SPC):
                for b in range(n_bands):
                    r0 = b * 256
                    acc = []  # accumulated gray tile per parity
                    for par in range(2):  # 0: even rows, 1: odd rows
                        ch_tiles = []
                        for ch in range(C):
                            t = in_pool.tile([128, W], f32, tag=f"in{par}{ch}")
                            nc.sync.dma_start(
                                out=t[:, :], in_=x[s, ch, r0 + par : r0 + 256 : 2, :]
                            )
                            ch_tiles.append(t)
                        g = ch_tiles[0]
                        nc.vector.scalar_tensor_tensor(
                            g[:, :], ch_tiles[1][:, :], r1, g[:, :], mult, add
                        )
                        nc.vector.scalar_tensor_tensor(
                            g[:, :], ch_tiles[2][:, :], r2, g[:, :], mult, add
                        )
                        nc.scalar.mul(g[:, :], g[:, :], w0h)
                        acc.append(g)
                    gE, gO = acc
                    drow = mid_pool.tile([128, W], f32, tag="drow")
                    nc.vector.tensor_tensor(drow[:, :], gE[:, :], gO[:, :], sub)
                    # srow overwrites gE (WAR on drow's read handled by Tile)
                    nc.vector.tensor_tensor(gE[:, :], gE[:, :], gO[:, :], add)
                    srow = gE

                    for sub_i, (src, op) in enumerate(
                        ((srow, add), (drow, add), (srow, sub), (drow, sub))
                    ):
                        # order: cA(srow,+), cH(drow,+), cV(srow,-), cD(drow,-)
                        o = out_pool.tile([128, WO], f32, tag=f"o{sub_i}")
                        nc.vector.tensor_tensor(
                            o[:, :], src[:, 0:W:2], src[:, 1:W:2], op
                        )
                        nc.sync.dma_start(
                            out=out[s, sub_i, b * 128 : b * 128 + 128, :], in_=o[:, :]
                        )
    nc.finalize()
    return nc


def kernel(x: np.ndarray) -> np.ndarray:
    global _compiled
    from concourse.bass_utils import run_bass_kernel_spmd

    if _compiled is None:
        _compiled = _build()
    nc = _compiled

    x = np.ascontiguousarray(x, dtype=np.float32)
    in_maps = [
        {"x": x[i * SPC : (i + 1) * SPC]} for i in range(N_CORES)
    ]
    res = run_bass_kernel_spmd(nc, in_maps, list(range(N_CORES))).results
    out = np.concatenate([r["out"] for r in res], axis=0)
    return out


# revision 6
# speedup vs baseline: 1.0993x; 1.0993x over previous
"""Grayscale + single-level 2x2 Haar DWT kernel for Trainium2 (8 cores, SPMD).

Full input x [16,3,1024,1024] f32 -> out [16,4,512,512] f32.
Batch-sharded: core i handles samples [2i, 2i+1].

Math per sample (BGR weights w=(0.114,0.587,0.299), all bands scaled by 0.5):
  gray = w0*x[0] + w1*x[1] + w2*x[2]
  a,b,c,d = gray[0::2,0::2], gray[0::2,1::2], gray[1::2,0::2], gray[1::2,1::2]
  cA,cH,cV,cD = 0.5*(a+b+c+d), 0.5*(a+b-c-d), 0.5*(a-b+c-d), 0.5*(a-b-c+d)

Per band of 128 output rows (= 256 input rows), all ops in-place where legal:
  E_ch/O_ch = even/odd input rows [128,1024] via strided DMA
  E0 += r1*E1 ; E0 += r2*E2  (r_i = w_i/w_0)    - scalar_tensor_tensor on DVE
  E0 *= w0/2   (ACT engine)                      - same for O0
  drow = E0 - O0 ; E0 += O0 (=srow)              - tensor_tensor on DVE
  cA/cV = E0[:,0::2] +/- E0[:,1::2] ; cH/cD = drow[:,0::2] +/- drow[:,1::2]
"""

import numpy as np

N_CORES = 8
B, C, H, W = 16, 3, 1024, 1024
HO, WO = H // 2, W // 2
SPC = B // N_CORES  # samples per core

W_BGR = (0.114, 0.587, 0.299)

_compiled = None


def _build():
    from concourse import bacc, mybir
    from concourse.tile import TileContext

    f32 = mybir.dt.float32
    add = mybir.AluOpType.add
    sub = mybir.AluOpType.subtract
    mult = mybir.AluOpType.mult

    w0, w1, w2 = W_BGR
    r1 = w1 / w0
    r2 = w2 / w0
    w0h = w0 * 0.5

    nc = bacc.Bacc("TRN2", target_bir_lowering=False, debug=False)
    x = nc.declare_dram_parameter("x", [SPC, C, H, W], f32, isOutput=False)
    out = nc.declare_dram_parameter("out", [SPC, 4, HO, WO], f32, isOutput=True)

    n_bands = H // 256  # bands of 128 output rows per sample

    with TileContext(nc) as tc:
        with (
            tc.tile_pool(name="in_pool", bufs=4) as in_pool,
            tc.tile_pool(name="mid_pool", bufs=4) as mid_pool,
            tc.tile_pool(name="out_pool", bufs=6) as out_pool,
        ):
            for s in range(SPC):
                for b in range(n_bands):
                    r0 = b * 256
                    acc = []  # accumulated (unscaled) gray tile per parity
                    for par in range(2):  # 0: even rows, 1: odd rows
                        ch_tiles = []
                        for ch in range(C):
                            t = in_pool.tile([128, W], f32, tag=f"in{par}{ch}")
                            nc.sync.dma_start(
                                out=t[:, :], in_=x[s, ch, r0 + par : r0 + 256 : 2, :]
                            )
                            ch_tiles.append(t)
                        g = ch_tiles[0]
                        nc.vector.scalar_tensor_tensor(
                            g[:, :], ch_tiles[1][:, :], r1, g[:, :], mult, add
                        )
                        nc.vector.scalar_tensor_tensor(
                            g[:, :], ch_tiles[2][:, :], r2, g[:, :], mult, add
                        )
                        acc.append(g)
                    gE, gO = acc
                    drow = mid_pool.tile([128, W], f32, tag="drow")
                    nc.vector.tensor_tensor(drow[:, :], gE[:, :], gO[:, :], sub)
                    # srow overwrites gE (WAR on drow's read handled by Tile)
                    nc.vector.tensor_tensor(gE[:, :], gE[:, :], gO[:, :], add)
                    srow = gE

                    for sub_i, (src, op) in enumerate(
                        ((srow, add), (drow, add), (srow, sub), (drow, sub))
                    ):
                        # order: cA(srow,+), cH(drow,+), cV(srow,-), cD(drow,-)
                        o = out_pool.tile([128, WO], f32, tag=f"o{sub_i}")
                        nc.vector.tensor_tensor(
                            o[:, :], src[:, 0:W:2], src[:, 1:W:2], op
                        )
                        # w0/2 scale applied in-place on ACT (downstream of
                        # DVE); store issued from the ACT HWDGE ring so loads
                        # (SP ring) and stores use separate FIFOs.
                        nc.scalar.mul(o[:, :], o[:, :], w0h)
                        nc.scalar.dma_start(
                            out=out[s, sub_i, b * 128 : b * 128 + 128, :], in_=o[:, :]
                        )
    nc.finalize()
    return nc


def kernel(x: np.ndarray) -> np.ndarray:
    global _compiled
    from concourse.bass_utils import run_bass_kernel_spmd

    if _compiled is None:
        _compiled = _build()
    nc = _compiled

    x = np.ascontiguousarray(x, dtype=np.float32)
    in_maps = [
        {"x": x[i * SPC : (i + 1) * SPC]} for i in range(N_CORES)
    ]
    res = run_bass_kernel_spmd(nc, in_maps, list(range(N_CORES))).results
    out = np.concatenate([r["out"] for r in res], axis=0)
    return out
